# revision 1
# baseline (speedup 1.0000x reference)
"""AxialClassifier kernel, v2: algebraically folded attention.

Per head:  dots_h = h @ A_h @ h^T   with A_h = Wq_h^T Wk_h * E**-0.5
           o      = sum_h attn_h @ h @ M_h^T + bo   with M_h = Wo_h Wv_h

This removes the separate q/k/v projections and head reshapes -> far
fewer HLO ops, and contraction always over D=8 or T=48 dense dims.
Data parallel over batch across the 8 NeuronCores, params replicated.
"""

import numpy as np

B, S, D, H, E, L = 128, 48, 8, 2, 4, 8
HD = H * E
NUM_CLASSES = 7
N_CORES = 8

_PARAM_NAMES = [
    "enc_w", "enc_b", "pos_row", "pos_col",
    "Wq", "Wk", "Wv", "Wo", "bo", "cls_w", "cls_b",
]

_STATE = {}


def _forward(x, enc_w, enc_b, pos_row, pos_col, Wq, Wk, Wv, Wo, bo, cls_w, cls_b):
    import jax
    import jax.numpy as jnp

    # Fold weights once (tiny).  A[l,a,h]: D x D ; M[l,a,h]: D x D
    Wq4 = Wq.reshape(L, 2, H, E, D)
    Wk4 = Wk.reshape(L, 2, H, E, D)
    Wv4 = Wv.reshape(L, 2, H, E, D)
    Wo4 = Wo.reshape(L, 2, D, H, E)
    A = jnp.einsum("lahed,lahef->lahdf", Wq4, Wk4) * (E**-0.5)  # (L,2,H,D,D)
    M = jnp.einsum("ladhe,lahef->lahdf", Wo4, Wv4)              # (L,2,H,D,D)

    def _attn(h, a_hd, m_hd, bo_):
        # h: (b, X, T, D) attend over T.  a_hd/m_hd: (H, D, D)
        g = jnp.einsum("bxtd,hdf->bxhtf", h, a_hd)             # (b,X,H,T,D)
        dots = jnp.einsum("bxhif,bxjf->bxhij", g, h)           # (b,X,H,T,T)
        attn = jax.nn.softmax(dots, axis=-1)
        u = jnp.einsum("bxhij,bxjf->bxhif", attn, h)           # (b,X,H,T,D)
        o = jnp.einsum("bxhif,hdf->bxid", u, m_hd)
        return o + bo_

    h = jnp.transpose(x, (0, 2, 3, 1))
    h = jax.nn.relu(h @ enc_w.T + enc_b)
    h = h + pos_row[None, :, None, :] + pos_col[None, None, :, :]
    for l in range(L):
        ht = jnp.transpose(h, (0, 2, 1, 3))
        o_row = jnp.transpose(_attn(ht, A[l, 0], M[l, 0], bo[l, 0]), (0, 2, 1, 3))
        o_col = _attn(h, A[l, 1], M[l, 1], bo[l, 1])
        h = jax.nn.relu(o_row + o_col)
    h = h.max(axis=-1)
    h = h.reshape(h.shape[0], -1)
    logits = h @ cls_w.T + cls_b
    return jax.nn.softmax(logits, axis=1)


def _setup():
    import jax
    from jax.sharding import Mesh, NamedSharding, PartitionSpec as P

    devs = jax.devices()[:N_CORES]
    mesh = Mesh(np.array(devs), axis_names=("dp",))
    batch_sh = NamedSharding(mesh, P("dp"))
    repl_sh = NamedSharding(mesh, P())
    fwd = jax.jit(
        _forward,
        in_shardings=tuple([batch_sh] + [repl_sh] * len(_PARAM_NAMES)),
        out_shardings=batch_sh,
    )
    _STATE["fwd"] = fwd
    _STATE["batch_sh"] = batch_sh
    _STATE["repl_sh"] = repl_sh
    return fwd


def kernel(**inputs) -> np.ndarray:
    import jax

    fwd = _STATE.get("fwd") or _setup()
    x = np.asarray(inputs["x"], dtype=np.float32)
    args = [jax.device_put(x, _STATE["batch_sh"])]
    for k in _PARAM_NAMES:
        args.append(
            jax.device_put(np.asarray(inputs[k], dtype=np.float32), _STATE["repl_sh"])
        )
    out = fwd(*args)
    return np.asarray(out).astype(np.float32)



# revision 2
# speedup vs baseline: 1.5458x; 1.5458x over previous
"""AxialClassifier kernel, v3: folded attention + minimal-round-trip plumbing.

The axon tunnel costs ~75-90ms per sequential round trip, so the warm
path is: (a) reuse resident device arrays for any input whose bytes are
unchanged (no re-upload), (b) one jitted dispatch, (c) replicated output
so np.asarray is a single-device fetch.
"""

import numpy as np

B, S, D, H, E, L = 128, 48, 8, 2, 4, 8
HD = H * E
NUM_CLASSES = 7
N_CORES = 8

_PARAM_NAMES = [
    "enc_w", "enc_b", "pos_row", "pos_col",
    "Wq", "Wk", "Wv", "Wo", "bo", "cls_w", "cls_b",
]

_STATE = {}


def _forward(x, enc_w, enc_b, pos_row, pos_col, Wq, Wk, Wv, Wo, bo, cls_w, cls_b):
    import jax
    import jax.numpy as jnp

    Wq4 = Wq.reshape(L, 2, H, E, D)
    Wk4 = Wk.reshape(L, 2, H, E, D)
    Wv4 = Wv.reshape(L, 2, H, E, D)
    Wo4 = Wo.reshape(L, 2, D, H, E)
    A = jnp.einsum("lahed,lahef->lahdf", Wq4, Wk4) * (E**-0.5)  # (L,2,H,D,D)
    M = jnp.einsum("ladhe,lahef->lahdf", Wo4, Wv4)              # (L,2,H,D,D)

    def _attn(h, a_hd, m_hd, bo_):
        # h: (b, X, T, D) attend over T.  a_hd/m_hd: (H, D, D)
        g = jnp.einsum("bxtd,hdf->bxhtf", h, a_hd)             # (b,X,H,T,D)
        dots = jnp.einsum("bxhif,bxjf->bxhij", g, h)           # (b,X,H,T,T)
        attn = jax.nn.softmax(dots, axis=-1)
        u = jnp.einsum("bxhij,bxjf->bxhif", attn, h)           # (b,X,H,T,D)
        o = jnp.einsum("bxhif,hdf->bxid", u, m_hd)
        return o + bo_

    h = jnp.transpose(x, (0, 2, 3, 1))
    h = jax.nn.relu(h @ enc_w.T + enc_b)
    h = h + pos_row[None, :, None, :] + pos_col[None, None, :, :]
    for l in range(L):
        ht = jnp.transpose(h, (0, 2, 1, 3))
        o_row = jnp.transpose(_attn(ht, A[l, 0], M[l, 0], bo[l, 0]), (0, 2, 1, 3))
        o_col = _attn(h, A[l, 1], M[l, 1], bo[l, 1])
        h = jax.nn.relu(o_row + o_col)
    h = h.max(axis=-1)
    h = h.reshape(h.shape[0], -1)
    logits = h @ cls_w.T + cls_b
    return jax.nn.softmax(logits, axis=1)


def _setup():
    import jax
    from jax.sharding import Mesh, NamedSharding, PartitionSpec as P

    devs = jax.devices()[:N_CORES]
    mesh = Mesh(np.array(devs), axis_names=("dp",))
    batch_sh = NamedSharding(mesh, P("dp"))
    repl_sh = NamedSharding(mesh, P())
    fwd = jax.jit(
        _forward,
        in_shardings=tuple([batch_sh] + [repl_sh] * len(_PARAM_NAMES)),
        out_shardings=repl_sh,
    )
    _STATE["fwd"] = fwd
    _STATE["batch_sh"] = batch_sh
    _STATE["repl_sh"] = repl_sh
    _STATE["res"] = {}
    return fwd


def _resident(name, value, sharding):
    """Device-resident cache: re-upload only when the bytes changed."""
    import jax

    value = np.asarray(value, dtype=np.float32)
    ent = _STATE["res"].get(name)
    if ent is not None and ent[0].shape == value.shape and np.array_equal(ent[0], value):
        return ent[1]
    dev = jax.device_put(value, sharding)
    _STATE["res"][name] = (value.copy(), dev)
    return dev


def kernel(**inputs) -> np.ndarray:
    fwd = _STATE.get("fwd") or _setup()
    args = [_resident("x", inputs["x"], _STATE["batch_sh"])]
    for k in _PARAM_NAMES:
        args.append(_resident(k, inputs[k], _STATE["repl_sh"]))
    out = fwd(*args)
    return np.asarray(out).astype(np.float32)
